# revision 1
# baseline (speedup 1.0000x reference)
"""AAUpsample1d Trainium2 kernel (fp8 DoubleRow edition).

Reference computation (per batch element):
  y   = conv_transpose1d(x, conv_w, stride=2, k=3) + conv_b        # [512, 8192]
  y   = depthwise_conv1d(y, aa_kernel, k=17, same)                 # anti-alias
  out = proj_w @ y + proj_b                                        # 1x1 projection

Algebraic restructuring (same as the bf16 baseline):
  * depthwise AA commutes with the 1x1 projection; fold proj into the three
    polyphase matrices M_k = proj_w @ conv_w[:,:,k]:
        z[2u]   = M1 @ x[u]
        z[2u+1] = M0 @ x[u] + M2 @ x[u+1]
        out     = AA(z) + (sum(aa) * proj_w @ conv_b + proj_b)
  * AA(z) runs on the TensorEngine as banded-Toeplitz matmuls with z produced
    length-on-partitions (overlapping 128-u tiles advancing by 120).

fp8 acceleration:
  * All matmuls use MatmulPerfMode.DoubleRow (fp8, two K=128 tiles per pass,
    0.5 PE cycles per output column = 4x the bf16 MAC rate).
  * Accuracy is held with hi/lo splitting: A ~ e4m3(A) + e5m2(A - e4m3(A)).
    e5m2 for the lo part keeps its tiny values out of the fp8 subnormal range
    with NO extra scale, so hi and lo products share one PSUM accumulation
    group.  z = xh@Mh + xl@Mh + xh@Ml  (the xl@Ml term is O(delta^2), dropped).
  * M is pre-scaled by SM=1024 so e4m3(SM*M) stays in the e4m3 normal range;
    the 1/SM is applied in the PSUM->SBUF z copies (activation scale).
  * The AA Toeplitz matrix is quantized to e4m3 at a host-optimized global
    scale SR (undone in the output copies); z enters AA as zhi(e4m3)+zlo(e5m2),
    one DoubleRow pass each with phase pairs (ze,zo) in the two K slots.
  * The walrus dual-fp8 Ldweights ISA check requires the stationary operand's
    two K slots to be one dense 256-element run per partition, so x is
    pre-windowed on the host into [p, J, q, tap, slot, 128] (per z tile J,
    ic-chunk-pair q, conv tap shift, ic-chunk slot) and the z copies write
    [p, cc, phase, 128] so AA lhsT slices are dense too.
  * Output is written bf16 and widened to f32 on host.

Sharding: pure data-parallel, one batch element per NeuronCore (B=8 = n_cores).
"""

import numpy as np
import ml_dtypes

import concourse.bass as bass
import concourse.mybir as mybir
import concourse.tile as tile
from concourse import bacc
from concourse.bass_utils import run_bass_kernel_spmd

BF16 = ml_dtypes.bfloat16
E4 = ml_dtypes.float8_e4m3
E5 = ml_dtypes.float8_e5m2

B, D, L = 8, 512, 4096
LOUT = 2 * L
KSIZE = 17
STRIDE = 120                      # u-positions of fresh output per z tile
NJ = (L + STRIDE - 1) // STRIDE   # 35 tiles
RCOLS = 2 * STRIDE                # 240 interleaved output columns per tile
XPAD_L = 4                        # left zero pad (covers u < 0 halo)
XCOLS = XPAD_L + L + 132          # 4232: right pad so every slice is in-bounds
NCC = D // 128                    # 4 channel chunks
NQ = 2                            # DoubleRow ic-chunk pairs per 512 contraction
SM = 1024.0                       # e4m3 scale for the M matrices
XWB = NQ * 2 * 2 * 128            # bytes/partition of windowed x per tile J
# x segments: smaller first segment shrinks the bytes gating J0's completion;
# fine-grained later segments interleave with output-DMA bursts on the shared
# DMA engines so x never falls behind the compute wavefront
SEGSTARTS = [0, 2, 5, 8, 11, 14, 17, 20, 23, 26, 29, 32]
NSEG = len(SEGSTARTS)
_SEGENDS = SEGSTARTS[1:] + [NJ]

_CACHE = {}


def _build_bass(srinv):
    nc = bacc.Bacc("TRN2", target_bir_lowering=False)
    f32 = mybir.dt.float32
    bf16 = mybir.dt.bfloat16
    e4 = mybir.dt.float8e4
    e5 = mybir.dt.float8e5
    DR = mybir.MatmulPerfMode.DoubleRow
    Ident = mybir.ActivationFunctionType.Identity

    # x pre-windowed: [p, J, q, tap, slot, m]; value = x[(2q+s)*128+p, 120J+t+m]
    xh_d = nc.dram_tensor("xh", [128, NJ, NQ, 2, 2, 128], e4, kind="ExternalInput")
    xl_d = nc.dram_tensor("xl", [128, NJ, NQ, 2, 2, 128], e5, kind="ExternalInput")
    mth_d = nc.dram_tensor("mth", [D, 3, D], e4, kind="ExternalInput")
    mtl_d = nc.dram_tensor("mtl", [D, 3, D], e5, kind="ExternalInput")
    rt_d = nc.dram_tensor("rt", [128, 2, RCOLS], e4, kind="ExternalInput")
    rtb_d = nc.dram_tensor("rtb", [128, 2, RCOLS], bf16, kind="ExternalInput")
    bias_d = nc.dram_tensor("bias", [D], f32, kind="ExternalInput")
    out_d = nc.dram_tensor("out", [D, LOUT], bf16, kind="ExternalOutput")

    with tile.TileContext(nc) as tc:
        with (
            tc.tile_pool(name="const", bufs=1) as cpool,
            tc.tile_pool(name="zhi", bufs=2) as zhpool,
            tc.tile_pool(name="zlo", bufs=2) as zlpool,
            tc.tile_pool(name="osb", bufs=3) as opool,
            tc.tile_pool(name="zmm", bufs=4, space="PSUM") as zmm,
            tc.tile_pool(name="aa", bufs=4, space="PSUM") as aamm,
        ):
            # ---- constants / inputs ----
            # DMA emission order = execution order on the shared DMA engines:
            # hi weights + first hi-x segment first so the first matmul
            # unblocks early; lo tensors next; then stream remaining segments.
            mtsh = [cpool.tile([128, 2, 3, D], e4, name=f"mtsh{q}")
                    for q in range(NQ)]
            mtsl = [cpool.tile([128, 2, 3, D], e5, name=f"mtsl{q}")
                    for q in range(NQ)]
            seglen = [_SEGENDS[s] - SEGSTARTS[s] for s in range(NSEG)]
            xhsegs = [cpool.tile([128, seglen[s], NQ, 2, 2, 128], e4,
                                 name=f"xhseg{s}") for s in range(NSEG)]
            xlsegs = [cpool.tile([128, seglen[s], NQ, 2, 2, 128], e5,
                                 name=f"xlseg{s}") for s in range(NSEG)]

            # arrival order gates the first tiles' products: hi weights + hi x
            # first (hi.hi), then lo weights (hi.lo), then lo x (lo.hi)
            def _dma_mts(dst, src_d, q):
                src = src_d[256 * q:256 * (q + 1)]
                nc.sync.dma_start(dst[q][:], src.rearrange("(s p) k o -> p s k o", p=128))

            _dma_mts(mtsh, mth_d, 0)
            nc.sync.dma_start(xhsegs[0][:], xh_d[:, SEGSTARTS[0]:_SEGENDS[0]])
            _dma_mts(mtsh, mth_d, 1)
            _dma_mts(mtsl, mtl_d, 0)
            _dma_mts(mtsl, mtl_d, 1)
            nc.sync.dma_start(xlsegs[0][:], xl_d[:, SEGSTARTS[0]:_SEGENDS[0]])

            rts = cpool.tile([128, 2, RCOLS], e4, name="rts")
            nc.sync.dma_start(rts[:], rt_d[:])
            rtsb = cpool.tile([128, 2, RCOLS], bf16, name="rtsb")
            nc.sync.dma_start(rtsb[:], rtb_d[:])
            biast = cpool.tile([128, NCC], f32, name="biast")
            nc.sync.dma_start(biast[:], bias_d.rearrange("(o p) -> p o", p=128))

            # all x segments up front: a queued output DMA waiting on its
            # copies would otherwise head-of-line-block later x segments in
            # the DGE queue, starving the compute wavefront mid-kernel
            for s in range(1, NSEG):
                nc.sync.dma_start(xhsegs[s][:], xh_d[:, SEGSTARTS[s]:_SEGENDS[s]])
                nc.sync.dma_start(xlsegs[s][:], xl_d[:, SEGSTARTS[s]:_SEGENDS[s]])

            zs = [None] * NJ
            pair_psum = {}
            out_r = out_d.rearrange("(cc p) l -> p cc l", p=128)

            # warm-up matmuls on a zeroed scratch tile: ramps the PE clock
            # (HAM / p-state) out of its cold state while the first DMAs land
            NWARM = 56
            wsb = cpool.tile([128, 64], bf16, name="wsb")
            nc.vector.memset(wsb[:], 0.0)
            wps = zmm.tile([128, D], f32, tag="zmm", name="wps")
            for _ in range(NWARM):
                nc.tensor.matmul(wps[:64, :64], lhsT=wsb[:], rhs=wsb[:],
                                 start=True, stop=True)

            seg_of = {J: s for s in range(NSEG)
                      for J in range(SEGSTARTS[s], _SEGENDS[s])}

            def emit_main(J):
                # z tile J covers u in [120J-4, 120J+124)
                s = seg_of[J]
                Js = J - SEGSTARTS[s]
                xh_t, xl_t = xhsegs[s], xlsegs[s]
                # 18 DoubleRow matmuls: products grouped to match DMA arrival
                # order (mtsh q0, xh, mtsh q1, mtl, xl) so the first tiles
                # start as soon as each operand lands.  ze and zo accumulate
                # in two concurrently-open PSUM groups.
                ze_ps = zmm.tile([128, D], f32, tag="zmm", name="ze_ps")
                zo_ps = zmm.tile([128, D], f32, tag="zmm", name="zo_ps")
                ne = no = 0
                for xt, mts in ((xh_t, mtsh), (xh_t, mtsl), (xl_t, mtsh)):
                    for q in range(NQ):
                        nc.tensor.matmul(
                            ze_ps, lhsT=xt[:, Js, q, 0],
                            rhs=mts[q][:, :, 1], perf_mode=DR,
                            start=(ne == 0), stop=(ne == 3 * NQ - 1),
                        )
                        ne += 1
                        for k, tap in ((0, 0), (2, 1)):
                            nc.tensor.matmul(
                                zo_ps, lhsT=xt[:, Js, q, tap],
                                rhs=mts[q][:, :, k], perf_mode=DR,
                                start=(no == 0), stop=(no == 6 * NQ - 1),
                            )
                            no += 1
                # PSUM -> SBUF quantized copies, into the AA-ready interleaved
                # layout [p, cc, phase, m].
                ze_v = ze_ps[:].rearrange("p (c m) -> p c m", c=NCC)
                zo_v = zo_ps[:].rearrange("p (c m) -> p c m", c=NCC)
                if J == NJ - 1:
                    # tail fast-path: these copies gate the kernel's final AA
                    # matmuls, so do single bf16 copies on two engines in
                    # parallel (the hi/lo chain would serialize ~1.9us)
                    zbf = zhpool.tile([128, NCC, 2, 128], bf16, tag="zhi",
                                      name="zbf")
                    nc.scalar.activation(zbf[:, :, 0], ze_v, Ident,
                                         scale=1.0 / SM)
                    nc.vector.tensor_scalar_mul(zbf[:, :, 1], zo_v, 1.0 / SM)
                    zs[J] = (zbf, None)
                    return
                # zhi = e4m3(psum/SM) on ACT; zlo = e5m2(psum/SM - zhi) on DVE.
                zhi = zhpool.tile([128, NCC, 2, 128], e4, tag="zhi", name="zhi")
                zlo = zlpool.tile([128, NCC, 2, 128], e5, tag="zlo", name="zlo")
                nc.scalar.activation(zhi[:, :, 0], ze_v, Ident, scale=1.0 / SM)
                nc.scalar.activation(zhi[:, :, 1], zo_v, Ident, scale=1.0 / SM)
                nc.vector.scalar_tensor_tensor(
                    out=zlo[:, :, 0], in0=ze_v, scalar=1.0 / SM,
                    in1=zhi[:, :, 0], op0=mybir.AluOpType.mult,
                    op1=mybir.AluOpType.subtract,
                )
                nc.vector.scalar_tensor_tensor(
                    out=zlo[:, :, 1], in0=zo_v, scalar=1.0 / SM,
                    in1=zhi[:, :, 1], op0=mybir.AluOpType.mult,
                    op1=mybir.AluOpType.subtract,
                )
                zs[J] = (zhi, zlo)

            # out-tile grouping: pairs of J share one PSUM bank row; the last
            # one goes solo so its copies+DMA overlap the remaining PE work
            PAIRS = [(a, a + 1) for a in range(0, NJ - 1, 2)] + [(NJ - 1,)]
            pair_of = {J: (p, grp.index(J), grp) for p, grp in enumerate(PAIRS)
                       for J in grp}

            def emit_aa(J):
                p, half, grp = pair_of[J]
                if half == 0:
                    pair_psum[p] = [
                        aamm.tile([128, 512], f32, tag="aa", name=f"aa_ps{cc}")
                        for cc in range(NCC)
                    ]
                # last tile: only 32 of the 240 interleaved out cols are real
                mmcols = min(RCOLS, LOUT - RCOLS * J)
                zhi, zlo = zs[J]
                for cc in range(NCC):
                    dst = pair_psum[p][cc][:, RCOLS * half:RCOLS * half + mmcols]
                    if zlo is None:          # bf16 tail path
                        nc.tensor.matmul(
                            dst, lhsT=zhi[:, cc, 0], rhs=rtsb[:, 0, :mmcols],
                            start=True, stop=False,
                        )
                        nc.tensor.matmul(
                            dst, lhsT=zhi[:, cc, 1], rhs=rtsb[:, 1, :mmcols],
                            start=False, stop=True,
                        )
                        continue
                    nc.tensor.matmul(
                        dst, lhsT=zhi[:, cc], rhs=rts[:, :, :mmcols],
                        perf_mode=DR, start=True, stop=False,
                    )
                    nc.tensor.matmul(
                        dst, lhsT=zlo[:, cc], rhs=rts[:, :, :mmcols],
                        perf_mode=DR, start=False, stop=True,
                    )
                zs[J] = None
                if half == len(grp) - 1:
                    lbase = RCOLS * grp[0]
                    ncols = min(RCOLS * len(grp), LOUT - lbase)
                    osb = opool.tile([128, NCC, 2 * RCOLS], bf16,
                                     tag="osb", name="osb")
                    for cc in range(NCC):
                        # out = psum/SR + bias; GPSIMD can't read PSUM, so
                        # split 3:1 ACT:DVE — DVE's zlo subtracts gate main
                        # PSUM reuse, so keep DVE the lighter engine
                        if cc != 3:
                            nc.scalar.activation(
                                osb[:, cc, :ncols], pair_psum[p][cc][:, :ncols],
                                Ident, bias=biast[:, cc:cc + 1],
                                scale=srinv,
                            )
                        else:
                            nc.vector.scalar_tensor_tensor(
                                out=osb[:, cc, :ncols],
                                in0=pair_psum[p][cc][:, :ncols],
                                scalar=srinv,
                                in1=biast[:, cc:cc + 1].to_broadcast((128, ncols)),
                                op0=mybir.AluOpType.mult, op1=mybir.AluOpType.add,
                            )
                    # two half-DMAs: shorter bursts interleave more fairly
                    # with the x-segment DMAs on the shared DMA engines
                    nc.sync.dma_start(
                        out_r[:, 0:2, lbase:lbase + ncols], osb[:, 0:2, :ncols]
                    )
                    nc.sync.dma_start(
                        out_r[:, 2:4, lbase:lbase + ncols], osb[:, 2:4, :ncols]
                    )
                    del pair_psum[p]

            # software-pipelined emission: AA(J-1) after main(J) so the PE
            # never waits on the z copies.  The skew collapses at the last J
            # so the final pair's copies + out DMAs overlap main(NJ-1).
            for J in range(NJ - 1):
                emit_main(J)
                if J >= 1:
                    emit_aa(J - 1)
            emit_aa(NJ - 2)
            emit_main(NJ - 1)
            emit_aa(NJ - 1)

    nc.compile()
    return nc


def _host_weights(conv_w, conv_b, aa_kernel, proj_w, proj_b):
    aa = np.asarray(aa_kernel, np.float32)
    proj_w = np.asarray(proj_w, np.float32)
    # fold the projection into the three polyphase matrices, hi/lo split
    m = [proj_w @ np.asarray(conv_w, np.float32)[:, :, k] for k in range(3)]
    mh = [(mk * SM).astype(E4) for mk in m]
    ml = [(mk * SM - mhk.astype(np.float32)).astype(E5)
          for mk, mhk in zip(m, mh)]
    mth_np = np.ascontiguousarray(np.stack([mk.T for mk in mh], axis=1))
    mtl_np = np.ascontiguousarray(np.stack([mk.T for mk in ml], axis=1))

    # global scale for the e4m3 AA taps, optimized against the actual kernel
    nz = aa[np.abs(aa) > 1e-9]
    best = (np.inf, 1.0)
    for s in np.geomspace(0.5, 4.0, 8001):
        q = (nz * s).astype(E4).astype(np.float32) / s
        err = float(np.sum((q - nz) ** 2))
        if err < best[0]:
            best = (err, float(s))
    sr = best[1]

    u = np.arange(128)[:, None]
    l = np.arange(RCOLS)[None, :]
    te = 2 * u - l
    to = 2 * u - l + 1
    r_e = np.where((te >= 0) & (te < KSIZE),
                   sr * aa[np.clip(te, 0, KSIZE - 1)], 0.0)
    r_o = np.where((to >= 0) & (to < KSIZE),
                   sr * aa[np.clip(to, 0, KSIZE - 1)], 0.0)
    rt_np = np.stack([r_e, r_o], axis=1).astype(E4)   # [128, 2, 240]
    rtb_np = np.stack([r_e, r_o], axis=1).astype(BF16)  # bf16 twin for the tail

    bias_np = (aa.sum() * (proj_w @ np.asarray(conv_b, np.float32))
               + np.asarray(proj_b, np.float32)).astype(np.float32)
    return mth_np, mtl_np, rt_np, rtb_np, bias_np, 1.0 / sr


def _window_x(xpad):
    """[B, D, XCOLS] (fp8) -> [B, 128, NJ, NQ, 2, 2, 128] windowed layout."""
    # value[b, p, J, q, tap, slot, m] = xpad[b, (2q+slot)*128 + p, 120J+tap+m]
    dt = xpad.dtype
    xc = xpad.view(np.uint8).reshape(B, NQ, 2, 128, XCOLS)  # [b, q, slot, p, col]
    win = np.lib.stride_tricks.sliding_window_view(
        xc, 128, axis=-1)                              # [b, q, slot, p, c0, m]
    c0 = (STRIDE * np.arange(NJ)[:, None] + np.arange(2)[None, :])  # [J, tap]
    w = win[:, :, :, :, c0]                            # [b, q, slot, p, J, tap, m]
    return np.ascontiguousarray(w.transpose(0, 3, 4, 1, 5, 2, 6)).view(dt)


def kernel(x, conv_w, conv_b, aa_kernel, proj_w, proj_b):
    mth_np, mtl_np, rt_np, rtb_np, bias_np, srinv = _host_weights(
        conv_w, conv_b, aa_kernel, proj_w, proj_b)
    if "nc" not in _CACHE:
        _CACHE["nc"] = _build_bass(srinv)
    nc = _CACHE["nc"]

    x = np.asarray(x, np.float32)
    xh = np.zeros((B, D, XCOLS), E4)
    xl = np.zeros((B, D, XCOLS), E5)
    xh[:, :, XPAD_L:XPAD_L + L] = x.astype(E4)
    xl[:, :, XPAD_L:XPAD_L + L] = (
        x - xh[:, :, XPAD_L:XPAD_L + L].astype(np.float32)).astype(E5)
    xh_w = _window_x(xh)
    xl_w = _window_x(xl)
    in_maps = [
        {"xh": xh_w[b], "xl": xl_w[b], "mth": mth_np, "mtl": mtl_np,
         "rt": rt_np, "rtb": rtb_np, "bias": bias_np}
        for b in range(B)
    ]
    try:
        res = run_bass_kernel_spmd(nc, in_maps, core_ids=list(range(B)))
    except ModuleNotFoundError:
        # axon tunnel without NTFF profiling hooks + BASS_TRACE set in the
        # environment: retry untraced
        import os
        os.environ["BASS_NEVER_TRACE"] = "1"
        res = run_bass_kernel_spmd(nc, in_maps, core_ids=list(range(B)))
    _CACHE["last_results"] = res
    return np.stack([r["out"].astype(np.float32) for r in res.results], axis=0)

